# revision 11
# baseline (speedup 1.0000x reference)
"""Trainium2 Bass kernel for ChebyshevAdditiveAngularMargin loss.

Reference computation (per element of a [N, C] f32 matrix):
    cosine = clip(outputs, -1+eps, 1-eps)
    phi    = clenshaw(cosine, coeffs)            # degree-30 Chebyshev
    phi    = where(cosine > TH, phi, cosine - MM)
    out    = SCALE * (targets * phi + (1 - targets) * cosine)

`targets` is a one-hot matrix (one 1.0 per row), so out == SCALE*cosine
everywhere except a single element per row.  The kernel processes
4096-wide half-chunks of 128-row blocks:
  1. extract the chunk's hot cosine per row exactly with a fused
     multiply + row-sum on DVE (scalar_tensor_tensor accum_out; non-hot
     products are exactly 0.0 so the sum is exact).  A chunk without
     the hot column yields s=0; its correction is multiplied by the
     all-zero targets slice, so it vanishes and per-chunk corrections
     are safe.  The mandatory full-size product output goes to PSUM.
  2. clip in place on DVE (one dual-scalar-op 2x pass per chunk),
  3. run the exact 31-step Clenshaw recurrence batched over FOUR chunks
     (2 blocks x 2 halves, [128, 4] tiles) on DVE, matching jax's fp32
     op order — batching amortizes the ~150ns/op short-op overhead,
  4. scatter the correction back per chunk with one fused DVE op:
     cos += targets * delta[row, chunk],
  5. scale by 30 on the otherwise-idle ACT engine (off DVE's critical
     path: only the output DMA depends on it), then DMA out per chunk.

DVE does ~3 cycles/element (~230us/core); DMA moves 96 MB/core.
Rows are sharded across 8 NeuronCores (data parallel); the coefficient
vector is baked into the instruction stream as immediates (computed
from the runtime coeffs input).
"""

import sys

sys.path.insert(0, "/opt/trn_rl_repo")

import numpy as np

import concourse.bacc as bacc
import concourse.mybir as mybir
from concourse.tile import TileContext

F32 = mybir.dt.float32
OP = mybir.AluOpType
AF = mybir.ActivationFunctionType

N, C = 8192, 8192
N_CORES = 8
ROWS = N // N_CORES  # rows per core
P = 128  # SBUF partitions
PSUM_F = 4096  # PSUM free-dim capacity at f32 (2KB x 8 banks / 4B)
PAIR = 2  # blocks whose tiny-path is batched into one Clenshaw chain

MARGIN = 0.2
SCALE = 30.0
EPS = 1e-07
TH = float(np.cos(np.pi - MARGIN))
MM = float(np.sin(np.pi - MARGIN) * MARGIN)
CLIP_LO = float(np.float32(-1.0 + EPS))
CLIP_HI = float(np.float32(1.0 - EPS))


def build_bass(rows: int, cols: int, coeffs: np.ndarray):
    """Build the per-core program. Each core processes [rows, cols]."""
    cs = [float(c) for c in coeffs]  # f32 values, baked as immediates
    deg = len(cs) - 1
    n_blocks = rows // P
    fw = min(PSUM_F, cols)  # chunk width
    n_h = cols // fw  # chunks per block
    assert n_blocks % PAIR == 0
    n_w = PAIR * n_h  # chunks per chain batch

    nc = bacc.Bacc("TRN2", target_bir_lowering=False)
    x_d = nc.dram_tensor("outputs", [rows, cols], F32, kind="ExternalInput")
    t_d = nc.dram_tensor("targets", [rows, cols], F32, kind="ExternalInput")
    o_d = nc.dram_tensor("out", [rows, cols], F32, kind="ExternalOutput")

    with TileContext(nc) as tc:
        with (
            tc.tile_pool(name="xp", bufs=3 * n_h) as xp,
            tc.tile_pool(name="tp", bufs=2 * n_h + 1) as tp,
            tc.tile_pool(name="ps", bufs=1, space="PSUM") as sp,
            tc.tile_pool(name="tiny", bufs=2) as yp,
        ):
            scratch = sp.tile([P, fw], F32)  # extract's mandatory out
            for g in range(n_blocks // PAIR):
                sraw = yp.tile([P, n_w], F32, tag="sraw")
                xts, tts = [], []
                for j in range(n_w):  # chunk j = (block g*PAIR + j//n_h, half j%n_h)
                    r = slice((g * PAIR + j // n_h) * P, (g * PAIR + j // n_h + 1) * P)
                    cslice = slice((j % n_h) * fw, (j % n_h + 1) * fw)
                    xt = xp.tile([P, fw], F32, tag="xt")
                    tt = tp.tile([P, fw], F32, tag="tt")
                    nc.sync.dma_start(xt[:], x_d[r, cslice])
                    nc.sync.dma_start(tt[:], t_d[r, cslice])
                    xts.append(xt)
                    tts.append(tt)
                    # extract (DVE 1x): sraw[p,j] = sum_chunk targets*x
                    nc.vector.scalar_tensor_tensor(
                        scratch[:], tt[:], 1.0, xt[:], OP.mult, OP.mult,
                        accum_out=sraw[:, j : j + 1],
                    )
                    # clip in place (DVE 2x): cosine = clip(x)
                    nc.vector.tensor_scalar(
                        xt[:], xt[:], CLIP_HI, CLIP_LO, OP.min, OP.max
                    )

                # --- tiny path on DVE, [128, n_w] batched over chunks ---
                s = yp.tile([P, n_w], F32, tag="s")
                x2s = yp.tile([P, n_w], F32, tag="x2s")
                nc.vector.tensor_scalar(
                    s[:], sraw[:], CLIP_HI, CLIP_LO, OP.min, OP.max
                )
                nc.vector.tensor_scalar_mul(x2s[:], s[:], 2.0)

                b1 = yp.tile([P, n_w], F32, tag="b1")
                b2 = yp.tile([P, n_w], F32, tag="b2")
                bn = yp.tile([P, n_w], F32, tag="bn")
                tm = yp.tile([P, n_w], F32, tag="tm")
                nc.vector.memset(b1[:], cs[deg])  # step k=deg from (0,0)
                nc.vector.memset(b2[:], 0.0)
                for k in range(deg - 1, -1, -1):
                    # b_new = (c_k + x2*b1) - b2 rounded exactly like jax:
                    # tm = fl(x2*b1); bn = fl(fl(tm + c_k) - b2)
                    nc.vector.tensor_tensor(tm[:], x2s[:], b1[:], OP.mult)
                    nc.vector.scalar_tensor_tensor(
                        bn[:], tm[:], cs[k], b2[:], OP.add, OP.subtract
                    )
                    b1, b2, bn = bn, b1, b2
                # phi = b0 - b1*x  (post-loop: b0 is b1, b1 is b2)
                nc.vector.tensor_tensor(tm[:], b2[:], s[:], OP.mult)
                phi = yp.tile([P, n_w], F32, tag="phi")
                nc.vector.tensor_tensor(phi[:], b1[:], tm[:], OP.subtract)

                # phisel = where(s > TH, phi, s - MM); delta = phisel - s
                mask = yp.tile([P, n_w], F32, tag="mask")
                alt = yp.tile([P, n_w], F32, tag="alt")
                diff = yp.tile([P, n_w], F32, tag="diff")
                nc.vector.tensor_scalar(mask[:], s[:], TH, None, OP.is_gt)
                nc.vector.tensor_scalar_sub(alt[:], s[:], MM)
                nc.vector.tensor_tensor(diff[:], phi[:], alt[:], OP.subtract)
                phisel = yp.tile([P, n_w], F32, tag="phisel")
                nc.vector.tensor_tensor(phisel[:], diff[:], mask[:], OP.mult)
                nc.vector.tensor_tensor(phisel[:], phisel[:], alt[:], OP.add)
                delta = yp.tile([P, n_w], F32, tag="delta")
                nc.vector.tensor_tensor(delta[:], phisel[:], s[:], OP.subtract)

                # --- scatter + scale + out, per chunk
                for j in range(n_w):
                    r = slice((g * PAIR + j // n_h) * P, (g * PAIR + j // n_h + 1) * P)
                    cslice = slice((j % n_h) * fw, (j % n_h + 1) * fw)
                    nc.vector.scalar_tensor_tensor(
                        xts[j][:], tts[j][:], delta[:, j : j + 1], xts[j][:],
                        OP.mult, OP.add,
                    )
                    # scale *30 on ACT (only the output DMA depends on it)
                    nc.scalar.activation(
                        xts[j][:], xts[j][:], AF.Copy, bias=0.0, scale=SCALE
                    )
                    nc.sync.dma_start(o_d[r, cslice], xts[j][:])
    return nc


_TRACE = False  # test.py sets this to capture an NTFF profile
_LAST_RESULTS = None


def kernel(outputs: np.ndarray, targets: np.ndarray, coeffs: np.ndarray) -> np.ndarray:
    global _LAST_RESULTS
    from concourse.bass_utils import run_bass_kernel_spmd

    assert outputs.shape == (N, C) and targets.shape == (N, C)
    nc = build_bass(ROWS, C, np.asarray(coeffs))
    nc.finalize()
    in_maps = [
        {
            "outputs": np.ascontiguousarray(outputs[i * ROWS : (i + 1) * ROWS]),
            "targets": np.ascontiguousarray(targets[i * ROWS : (i + 1) * ROWS]),
        }
        for i in range(N_CORES)
    ]
    res = run_bass_kernel_spmd(
        nc, in_maps, core_ids=list(range(N_CORES)), trace=_TRACE
    )
    _LAST_RESULTS = res
    return np.concatenate([r["out"] for r in res.results], axis=0)


# revision 12
# speedup vs baseline: 1.0926x; 1.0926x over previous
"""Trainium2 Bass kernel for ChebyshevAdditiveAngularMargin loss.

Reference computation (per element of a [N, C] f32 matrix):
    cosine = clip(outputs, -1+eps, 1-eps)
    phi    = clenshaw(cosine, coeffs)            # degree-30 Chebyshev
    phi    = where(cosine > TH, phi, cosine - MM)
    out    = SCALE * (targets * phi + (1 - targets) * cosine)

`targets` is a one-hot matrix (one 1.0 per row), so out == SCALE*cosine
everywhere except a single element per row.  Per block of 128 rows:
  1. extract the hot cosine per row exactly with a fused multiply +
     row-sum on DVE (scalar_tensor_tensor accum_out; non-hot products
     are exactly 0.0 so the sum is exact).  The mandatory full-size
     product output goes to PSUM, so the extract runs per 4096-wide
     half; each half gets its own correction (a half without the hot
     column yields s=0 whose correction is multiplied by the all-zero
     targets slice, so it vanishes).
  2. clip on the otherwise-idle ACT engine as two in-place Relu passes:
     v = relu((hi-lo) - relu(hi - x)) == clip(x) - lo  (+-1 ulp),
  3. run the exact 31-step Clenshaw recurrence on the [128, 2] hot
     values on DVE (matching jax's fp32 op order) using the exactly
     clipped extract (tiny dual-scalar-op clip),
  4. scatter the correction back per half with one fused DVE op:
     v += targets * delta[row,h]   (delta = phisel - s),
  5. final ACT pass folds the +lo back in while scaling:
     out = Copy(v*30 + 30*lo) == SCALE*(v + lo), then DMA out.

DVE does ~2 cycles/element (~220us/core: extract + scatter at 1x plus
~10us/block of short Clenshaw ops); ACT does 3 big passes (~170us);
DMA moves 96 MB/core (~240us of queue busy).  Rows are sharded across
8 NeuronCores (data parallel); the coefficient vector is baked into
the instruction stream as immediates (from the runtime coeffs input).
"""

import sys

sys.path.insert(0, "/opt/trn_rl_repo")

import numpy as np

import concourse.bacc as bacc
import concourse.mybir as mybir
from concourse.tile import TileContext

F32 = mybir.dt.float32
OP = mybir.AluOpType
AF = mybir.ActivationFunctionType

N, C = 8192, 8192
N_CORES = 8
ROWS = N // N_CORES  # rows per core
P = 128  # SBUF partitions
PSUM_F = 4096  # PSUM free-dim capacity at f32 (2KB x 8 banks / 4B)

MARGIN = 0.2
SCALE = 30.0
EPS = 1e-07
TH = float(np.cos(np.pi - MARGIN))
MM = float(np.sin(np.pi - MARGIN) * MARGIN)
CLIP_LO = float(np.float32(-1.0 + EPS))
CLIP_HI = float(np.float32(1.0 - EPS))
CLIP_SPAN = float(np.float32(CLIP_HI) - np.float32(CLIP_LO))  # hi - lo
BIAS30LO = float(np.float32(SCALE) * np.float32(CLIP_LO))  # fl(30*lo)


def build_bass(rows: int, cols: int, coeffs: np.ndarray):
    """Build the per-core program. Each core processes [rows, cols]."""
    cs = [float(c) for c in coeffs]  # f32 values, baked as immediates
    deg = len(cs) - 1
    n_blocks = rows // P
    fw = min(PSUM_F, cols)  # extract half width
    n_h = cols // fw  # halves per block

    nc = bacc.Bacc("TRN2", target_bir_lowering=False)
    x_d = nc.dram_tensor("outputs", [rows, cols], F32, kind="ExternalInput")
    t_d = nc.dram_tensor("targets", [rows, cols], F32, kind="ExternalInput")
    o_d = nc.dram_tensor("out", [rows, cols], F32, kind="ExternalOutput")

    with TileContext(nc) as tc:
        with (
            tc.tile_pool(name="xp", bufs=3) as xp,
            tc.tile_pool(name="tp", bufs=2) as tp,
            tc.tile_pool(name="ps", bufs=1, space="PSUM") as sp,
            tc.tile_pool(name="cst", bufs=1) as cp,
            tc.tile_pool(name="tiny", bufs=2) as yp,
        ):
            scratch = sp.tile([P, fw], F32)  # extract's mandatory out
            chi = cp.tile([P, 1], F32)  # Relu biases must be APs
            cspan = cp.tile([P, 1], F32)
            nc.vector.memset(chi[:], CLIP_HI)
            nc.vector.memset(cspan[:], CLIP_SPAN)
            for b in range(n_blocks):
                r = slice(b * P, (b + 1) * P)
                xt = xp.tile([P, cols], F32, tag="xt")
                tt = tp.tile([P, cols], F32, tag="tt")
                nc.sync.dma_start(xt[:], x_d[r, :])
                nc.sync.dma_start(tt[:], t_d[r, :])

                # --- extract (DVE 1x): sraw[p,h] = sum_half targets*x
                sraw = yp.tile([P, n_h], F32, tag="sraw")
                for h in range(n_h):
                    cslice = slice(h * fw, (h + 1) * fw)
                    nc.vector.scalar_tensor_tensor(
                        scratch[:], tt[:, cslice], 1.0, xt[:, cslice],
                        OP.mult, OP.mult,
                        accum_out=sraw[:, h : h + 1],
                    )

                # --- clip on ACT, in place: xt <- clip(x) - lo (+-1 ulp)
                nc.scalar.activation(xt[:], xt[:], AF.Relu, bias=chi[:], scale=-1.0)
                nc.scalar.activation(xt[:], xt[:], AF.Relu, bias=cspan[:], scale=-1.0)

                # --- tiny path on DVE, [128, n_h] batched over halves ---
                s = yp.tile([P, n_h], F32, tag="s")
                x2s = yp.tile([P, n_h], F32, tag="x2s")
                nc.vector.tensor_scalar(
                    s[:], sraw[:], CLIP_HI, CLIP_LO, OP.min, OP.max
                )
                nc.vector.tensor_scalar_mul(x2s[:], s[:], 2.0)

                b1 = yp.tile([P, n_h], F32, tag="b1")
                b2 = yp.tile([P, n_h], F32, tag="b2")
                bn = yp.tile([P, n_h], F32, tag="bn")
                tm = yp.tile([P, n_h], F32, tag="tm")
                nc.vector.memset(b1[:], cs[deg])  # step k=deg from (0,0)
                nc.vector.memset(b2[:], 0.0)
                for k in range(deg - 1, -1, -1):
                    # b_new = (c_k + x2*b1) - b2 rounded exactly like jax:
                    # tm = fl(x2*b1); bn = fl(fl(tm + c_k) - b2)
                    nc.vector.tensor_tensor(tm[:], x2s[:], b1[:], OP.mult)
                    nc.vector.scalar_tensor_tensor(
                        bn[:], tm[:], cs[k], b2[:], OP.add, OP.subtract
                    )
                    b1, b2, bn = bn, b1, b2
                # phi = b0 - b1*x  (post-loop: b0 is b1, b1 is b2)
                nc.vector.tensor_tensor(tm[:], b2[:], s[:], OP.mult)
                phi = yp.tile([P, n_h], F32, tag="phi")
                nc.vector.tensor_tensor(phi[:], b1[:], tm[:], OP.subtract)

                # phisel = where(s > TH, phi, s - MM); delta = phisel - s
                mask = yp.tile([P, n_h], F32, tag="mask")
                alt = yp.tile([P, n_h], F32, tag="alt")
                diff = yp.tile([P, n_h], F32, tag="diff")
                nc.vector.tensor_scalar(mask[:], s[:], TH, None, OP.is_gt)
                nc.vector.tensor_scalar_sub(alt[:], s[:], MM)
                nc.vector.tensor_tensor(diff[:], phi[:], alt[:], OP.subtract)
                phisel = yp.tile([P, n_h], F32, tag="phisel")
                nc.vector.tensor_tensor(phisel[:], diff[:], mask[:], OP.mult)
                nc.vector.tensor_tensor(phisel[:], phisel[:], alt[:], OP.add)
                delta = yp.tile([P, n_h], F32, tag="delta")
                nc.vector.tensor_tensor(delta[:], phisel[:], s[:], OP.subtract)

                # --- scatter (DVE 1x): v += targets * delta[row,h]
                for h in range(n_h):
                    cslice = slice(h * fw, (h + 1) * fw)
                    nc.vector.scalar_tensor_tensor(
                        xt[:, cslice], tt[:, cslice], delta[:, h : h + 1],
                        xt[:, cslice], OP.mult, OP.add,
                    )
                # --- out = SCALE*(v + lo) on ACT, then DMA out
                nc.scalar.activation(
                    xt[:], xt[:], AF.Copy, bias=BIAS30LO, scale=SCALE
                )
                nc.sync.dma_start(o_d[r, :], xt[:])
    return nc


_TRACE = False  # test.py sets this to capture an NTFF profile
_LAST_RESULTS = None


def kernel(outputs: np.ndarray, targets: np.ndarray, coeffs: np.ndarray) -> np.ndarray:
    global _LAST_RESULTS
    from concourse.bass_utils import run_bass_kernel_spmd

    assert outputs.shape == (N, C) and targets.shape == (N, C)
    nc = build_bass(ROWS, C, np.asarray(coeffs))
    nc.finalize()
    in_maps = [
        {
            "outputs": np.ascontiguousarray(outputs[i * ROWS : (i + 1) * ROWS]),
            "targets": np.ascontiguousarray(targets[i * ROWS : (i + 1) * ROWS]),
        }
        for i in range(N_CORES)
    ]
    res = run_bass_kernel_spmd(
        nc, in_maps, core_ids=list(range(N_CORES)), trace=_TRACE
    )
    _LAST_RESULTS = res
    return np.concatenate([r["out"] for r in res.results], axis=0)
